# revision 3
# baseline (speedup 1.0000x reference)
"""Causal self-attention (B=4, S=2048, E=1024, H=16, Dh=64) on 8 TRN2 NeuronCores.

Sharding: tensor-parallel over heads — core c owns heads {2c, 2c+1}, i.e. rows
[128c, 128c+128) of the Q/K/V projection output and the matching 128 input rows
of the out-projection. Each core computes a full-shape partial output; the host
sums the 8 partials (the all-reduce after out_proj).

Layout strategy on device (all matmuls in float32r — full-rate, ~1e-4 relative
error): everything is kept "feature-major" (the contraction dim on partitions)
so no fp32 transposes of activations are ever needed:
  qT/kT [d, s] <- wT-slices @ xT ;  scoresT [sk, sq] = kT.T-chunks @ qT ;
  attnT = exp(scoresT/8) * causal ;  yT_ext [65, sq] = [v | 1].T @ attnT
(the appended ones-column yields the softmax denominator in row 64);
out[s, e] = (yT/den).T @ woT  with woT pre-transposed on the host.
"""
import sys
import os

sys.path.insert(0, "/opt/trn_rl_repo")
os.environ.setdefault("JAX_COMPILATION_CACHE_DIR", "/tmp/jax_cache")

import numpy as np

B, S, E = 4, 2048, 1024
H, DH = 16, 64
NCORES = 8
DLOC = E // NCORES          # 128 head-dims per core (2 heads)
ST = 512                    # sq tile width
SKC = 128                   # sk chunk (psum partition dim)
NT = S // ST                # 4 sq tiles
NU = S // SKC               # 16 sk chunks
EO = E // 128               # 8 contraction chunks for projections

_compiled = None


def _build():
    import concourse.bacc as bacc
    import concourse.mybir as mybir
    import concourse.tile as tile
    from concourse.masks import make_identity

    F32R = mybir.dt.float32r
    F32 = mybir.dt.float32

    nc = bacc.Bacc("TRN2", target_bir_lowering=False, debug=False)

    xT_d = nc.dram_tensor("xT", [B, E, S], F32R, kind="ExternalInput")
    wqT_d = nc.dram_tensor("wqT", [E, DLOC], F32R, kind="ExternalInput")
    wkT_d = nc.dram_tensor("wkT", [E, DLOC], F32R, kind="ExternalInput")
    wvT_d = nc.dram_tensor("wvT", [E, DLOC], F32R, kind="ExternalInput")
    woT_d = nc.dram_tensor("woT", [DLOC, E], F32R, kind="ExternalInput")
    mask_d = nc.dram_tensor("mask", [128, 128], F32R, kind="ExternalInput")
    out_d = nc.dram_tensor("out", [B, S, E], F32, kind="ExternalOutput")

    with tile.TileContext(nc) as tc:
        with (
            tc.tile_pool(name="const", bufs=1) as cpool,
            tc.tile_pool(name="xp", bufs=1) as xpool,
            tc.tile_pool(name="actp", bufs=2) as apool,
            tc.tile_pool(name="attp", bufs=4) as attpool,
            tc.tile_pool(name="smallp", bufs=3) as spool,
            tc.tile_pool(name="outp", bufs=4) as opool,
            tc.tile_pool(name="ps_mm", bufs=2, space="PSUM") as ps_mm,
            tc.tile_pool(name="ps_sc", bufs=3, space="PSUM") as ps_sc,
            tc.tile_pool(name="ps_y", bufs=2, space="PSUM") as ps_y,
        ):
            # ---- persistent constants ----
            wq = cpool.tile([128, EO, DLOC], F32R, tag="wq")
            wk = cpool.tile([128, EO, DLOC], F32R, tag="wk")
            wv = cpool.tile([128, EO, DLOC], F32R, tag="wv")
            wo = cpool.tile([128, E], F32R, tag="wo")
            mask = cpool.tile([128, 128], F32R, tag="mask")
            nc.sync.dma_start(wq[:], wqT_d.rearrange("(eo p) d -> p eo d", p=128))
            nc.sync.dma_start(wk[:], wkT_d.rearrange("(eo p) d -> p eo d", p=128))
            nc.sync.dma_start(wv[:], wvT_d.rearrange("(eo p) d -> p eo d", p=128))
            nc.sync.dma_start(wo[:], woT_d[:])
            nc.sync.dma_start(mask[:], mask_d[:])
            ident32 = cpool.tile([128, 128], F32, tag="id32")
            make_identity(nc, ident32[:])
            ident = cpool.tile([128, 128], F32R, tag="id")
            nc.vector.tensor_copy(ident[:], ident32[:])
            ones16 = cpool.tile([128, NU], F32, tag="ones16")
            nc.vector.memset(ones16[:], 1.0)

            for b in range(B):
                xT = xpool.tile([128, EO, S], F32R, tag="xT")
                nc.sync.dma_start(xT[:], xT_d[b].rearrange("(eo p) s -> p eo s", p=128))

                # ---- projections (Dh-major) ----
                qT = apool.tile([DLOC, S], F32R, tag="qT")
                kT = apool.tile([DLOC, S], F32R, tag="kT")
                vT = apool.tile([DLOC, S], F32R, tag="vT")
                for w_sb, dst in ((wq, qT), (wk, kT), (wv, vT)):
                    for st in range(NT):
                        ps = ps_mm.tile([128, ST], F32, tag="mm")
                        for eo in range(EO):
                            nc.tensor.matmul(
                                ps[:], w_sb[:, eo], xT[:, eo, st * ST:(st + 1) * ST],
                                start=(eo == 0), stop=(eo == EO - 1),
                            )
                        nc.vector.tensor_copy(dst[:, st * ST:(st + 1) * ST], ps[:])

                # ---- V -> S-major with ones columns ----
                # v_sb[:, u, 0:64] = head0, col 64 = 1; [65:129] = head1, col 129 = 1
                v_sb = apool.tile([128, NU, 130], F32R, tag="v")
                nc.vector.tensor_copy(v_sb[:, :, 64], ones16[:])
                nc.vector.tensor_copy(v_sb[:, :, 129], ones16[:])
                for u in range(NU):
                    pst = ps_sc.tile([128, 128], F32R, tag="sc")
                    nc.tensor.transpose(pst[:], vT[:, u * 128:(u + 1) * 128], ident[:])
                    nc.vector.tensor_copy(v_sb[:, u, 0:64], pst[:, 0:64])
                    nc.vector.tensor_copy(v_sb[:, u, 65:129], pst[:, 64:128])

                # ---- attention ----
                yT = apool.tile([DLOC, S], F32R, tag="yT")
                for t in range(NT):
                    psy = [ps_y.tile([65, ST], F32, tag="y", name=f"y_{b}_{t}_{h}")
                           for h in range(2)]
                    for u in range(4 * t + 4):
                        diag = u >= 4 * t
                        rel = (u - 4 * t) * 128 if diag else 0
                        na = ST - rel
                        for h in range(2):
                            hs = slice(64 * h, 64 * h + 64)
                            pss = ps_sc.tile([128, ST], F32, tag="sc")
                            nc.tensor.matmul(
                                pss[:, :na],
                                kT[hs, u * 128:(u + 1) * 128],
                                qT[hs, t * ST + rel:(t + 1) * ST],
                                start=True, stop=True,
                            )
                            at = attpool.tile([128, ST], F32R, tag="at")
                            nc.scalar.activation(
                                at[:, :na], pss[:, :na],
                                mybir.ActivationFunctionType.Exp, scale=0.125,
                            )
                            if diag:
                                nc.vector.tensor_tensor(
                                    at[:, :128], at[:, :128], mask[:],
                                    mybir.AluOpType.mult,
                                )
                            nc.tensor.matmul(
                                psy[h][:, rel:], v_sb[:, u, 65 * h:65 * h + 65],
                                at[:, :na],
                                start=(u == 0), stop=(u == 4 * t + 3),
                            )
                    for h in range(2):
                        den = spool.tile([1, ST], F32, tag="den")
                        nc.vector.tensor_copy(den[:], psy[h][64:65, :])
                        dbc = spool.tile([64, ST], F32, tag="dbc")
                        nc.gpsimd.partition_broadcast(dbc[:], den[0:1, :])
                        rcp = spool.tile([64, ST], F32, tag="rcp")
                        nc.vector.reciprocal(rcp[:], dbc[:])
                        nc.vector.tensor_tensor(
                            yT[64 * h:64 * h + 64, t * ST:(t + 1) * ST],
                            psy[h][0:64, :], rcp[:], mybir.AluOpType.mult,
                        )

                    # ---- out projection for this sq tile ----
                    for si in range(4):
                        s0 = t * ST + si * 128
                        for eo2 in range(2):
                            pso = ps_mm.tile([128, 512], F32, tag="mm")
                            nc.tensor.matmul(
                                pso[:], yT[:, s0:s0 + 128],
                                wo[:, eo2 * 512:(eo2 + 1) * 512],
                                start=True, stop=True,
                            )
                            ob = opool.tile([128, 512], F32, tag="ob")
                            eng = nc.vector if (si + eo2) % 2 == 0 else nc.scalar
                            if eng is nc.vector:
                                nc.vector.tensor_copy(ob[:], pso[:])
                            else:
                                nc.scalar.activation(
                                    ob[:], pso[:],
                                    mybir.ActivationFunctionType.Copy,
                                )
                            nc.sync.dma_start(
                                out_d[b, s0:s0 + 128, eo2 * 512:(eo2 + 1) * 512],
                                ob[:],
                            )

    nc.compile()
    return nc


def _get_compiled():
    global _compiled
    if _compiled is None:
        _compiled = _build()
    return _compiled


def _prep_inputs(x, Wq, Wk, Wv, Wo):
    xT = np.ascontiguousarray(x.transpose(0, 2, 1), dtype=np.float32)
    mask = np.triu(np.ones((128, 128), np.float32))  # mask[p, j] = 1 iff j >= p
    in_maps = []
    for c in range(NCORES):
        r = slice(128 * c, 128 * c + 128)
        in_maps.append({
            "xT": xT,
            "wqT": np.ascontiguousarray(Wq[r, :].T, dtype=np.float32),
            "wkT": np.ascontiguousarray(Wk[r, :].T, dtype=np.float32),
            "wvT": np.ascontiguousarray(Wv[r, :].T, dtype=np.float32),
            "woT": np.ascontiguousarray(Wo[:, r].T, dtype=np.float32),
            "mask": mask,
        })
    return in_maps


def kernel(x, Wq, Wk, Wv, Wo):
    from concourse.bass_utils import run_bass_kernel_spmd

    nc = _get_compiled()
    in_maps = _prep_inputs(np.asarray(x), np.asarray(Wq), np.asarray(Wk),
                           np.asarray(Wv), np.asarray(Wo))
    res = run_bass_kernel_spmd(nc, in_maps, core_ids=list(range(NCORES)))
    out = np.zeros((B, S, E), np.float32)
    for c in range(NCORES):
        out += res.results[c]["out"]
    return out


# revision 10
# speedup vs baseline: 187.0602x; 187.0602x over previous
"""Causal self-attention (B=4, S=2048, E=1024, H=16, Dh=64) on 8 TRN2 NeuronCores.

Sharding: tensor-parallel over heads — core c owns heads {2c, 2c+1}, i.e. rows
[128c, 128c+128) of the Q/K/V projection output and the matching 128 input rows
of the out-projection. Each core computes a full-shape partial output; the host
sums the 8 partials (the all-reduce after out_proj).

Layout strategy on device (all matmuls in float32r — full-rate, ~1e-4 relative
error): everything is kept "feature-major" (the contraction dim on partitions)
so no fp32 transposes of activations are ever needed:
  qT/kT [d, s] <- wT-slices @ xT ;  scoresT [sk, sq] = kT.T-chunks @ qT ;
  attnT = exp(scoresT/8) * causal ;  yT_ext [65, sq] = [v | 1].T @ attnT
(the appended ones-column yields the softmax denominator in row 64);
out[s, e] = (yT/den).T @ woT  with woT pre-transposed on the host.
"""
import sys
import os

sys.path.insert(0, "/opt/trn_rl_repo")
os.environ.setdefault("JAX_COMPILATION_CACHE_DIR", "/tmp/jax_cache")

import numpy as np

B, S, E = 4, 2048, 1024
H, DH = 16, 64
NCORES = 8
DLOC = E // NCORES          # 128 head-dims per core (2 heads)
ST = 512                    # sq tile width
SKC = 128                   # sk chunk (psum partition dim)
NT = S // ST                # 4 sq tiles
NU = S // SKC               # 16 sk chunks
EO = E // 128               # 8 contraction chunks for projections

_compiled = {}


def _build(loop_n=1):
    import concourse.bacc as bacc
    import concourse.mybir as mybir
    import concourse.tile as tile
    from concourse.masks import make_identity

    F32R = mybir.dt.float32r
    F32 = mybir.dt.float32

    nc = bacc.Bacc("TRN2", target_bir_lowering=False, debug=False)

    xT_d = nc.dram_tensor("xT", [B, E, S], F32R, kind="ExternalInput")
    wqT_d = nc.dram_tensor("wqT", [E, DLOC], F32R, kind="ExternalInput")
    wkT_d = nc.dram_tensor("wkT", [E, DLOC], F32R, kind="ExternalInput")
    wvT_d = nc.dram_tensor("wvT", [E, DLOC], F32R, kind="ExternalInput")
    woT_d = nc.dram_tensor("woT", [DLOC, E], F32R, kind="ExternalInput")
    mask_d = nc.dram_tensor("mask", [128, 128], F32R, kind="ExternalInput")
    out_d = nc.dram_tensor("out", [B, S, E], F32, kind="ExternalOutput")

    with tile.TileContext(nc) as tc:
        with (
            tc.tile_pool(name="const", bufs=1) as cpool,
            tc.tile_pool(name="xp", bufs=1) as xpool,
            tc.tile_pool(name="actp", bufs=2) as apool,
            tc.tile_pool(name="attp", bufs=4) as attpool,
            tc.tile_pool(name="smallp", bufs=3) as spool,
            tc.tile_pool(name="outp", bufs=4) as opool,
            tc.tile_pool(name="ps_mm", bufs=2, space="PSUM") as ps_mm,
            tc.tile_pool(name="ps_sc", bufs=2, space="PSUM") as ps_sc,
            tc.tile_pool(name="ps_y", bufs=2, space="PSUM") as ps_y,
        ):
            # ---- persistent constants ----
            wq = cpool.tile([128, EO, DLOC], F32R, tag="wq")
            wk = cpool.tile([128, EO, DLOC], F32R, tag="wk")
            wv = cpool.tile([128, EO, DLOC], F32R, tag="wv")
            wo = cpool.tile([128, E], F32R, tag="wo")
            mask = cpool.tile([128, 128], F32R, tag="mask")
            nc.sync.dma_start(wq[:], wqT_d.rearrange("(eo p) d -> p eo d", p=128))
            nc.sync.dma_start(wk[:], wkT_d.rearrange("(eo p) d -> p eo d", p=128))
            nc.sync.dma_start(wv[:], wvT_d.rearrange("(eo p) d -> p eo d", p=128))
            nc.sync.dma_start(wo[:], woT_d[:])
            nc.sync.dma_start(mask[:], mask_d[:])
            ident32 = cpool.tile([128, 128], F32, tag="id32")
            make_identity(nc, ident32[:])
            ident = cpool.tile([128, 128], F32R, tag="id")
            nc.vector.tensor_copy(ident[:], ident32[:])
            ones16 = cpool.tile([128, NU], F32, tag="ones16")
            nc.vector.memset(ones16[:], 1.0)

            import contextlib
            loop_ctx = tc.For_i(0, loop_n, 1) if loop_n > 1 else contextlib.nullcontext()
            with loop_ctx:
                _emit_body(nc, tc, locals())

    nc.compile()
    return nc


def _emit_body(nc, tc, env):
    import concourse.mybir as mybir

    F32R = mybir.dt.float32r
    F32 = mybir.dt.float32
    (xT_d, out_d, xpool, apool, attpool, spool, opool, ps_mm, ps_sc, ps_y,
     wq, wk, wv, wo, mask, ident, ones16) = (
        env["xT_d"], env["out_d"], env["xpool"], env["apool"], env["attpool"],
        env["spool"], env["opool"], env["ps_mm"], env["ps_sc"], env["ps_y"],
        env["wq"], env["wk"], env["wv"], env["wo"], env["mask"], env["ident"],
        env["ones16"])
    if True:
            for b in range(B):
                xT = xpool.tile([128, EO, S], F32R, tag="xT")
                nc.sync.dma_start(xT[:], xT_d[b].rearrange("(eo p) s -> p eo s", p=128))

                # ---- projections (Dh-major) ----
                qT = apool.tile([DLOC, S], F32R, tag="qT")
                kT = apool.tile([DLOC, S], F32R, tag="kT")
                vT = apool.tile([DLOC, S], F32R, tag="vT")
                for w_sb, dst in ((wq, qT), (wk, kT), (wv, vT)):
                    for st in range(NT):
                        ps = ps_mm.tile([128, ST], F32, tag="mm")
                        for eo in range(EO):
                            nc.tensor.matmul(
                                ps[:], w_sb[:, eo], xT[:, eo, st * ST:(st + 1) * ST],
                                start=(eo == 0), stop=(eo == EO - 1),
                            )
                        nc.vector.tensor_copy(dst[:, st * ST:(st + 1) * ST], ps[:])

                # ---- V -> S-major with ones columns ----
                # v_sb[:, u, 0:64] = head0, col 64 = 1; [65:129] = head1, col 129 = 1
                v_sb = apool.tile([128, NU, 130], F32R, tag="v")
                nc.vector.tensor_copy(v_sb[:, :, 64], ones16[:])
                nc.vector.tensor_copy(v_sb[:, :, 129], ones16[:])
                for u in range(NU):
                    pst = ps_mm.tile([128, 128], F32R, tag="mm")
                    nc.tensor.transpose(pst[:], vT[:, u * 128:(u + 1) * 128], ident[:])
                    # one strided copy: [128, 2, 64] view of both head halves
                    nc.vector.tensor_copy(
                        v_sb[:, u].rearrange("p (two d) -> p two d", two=2)[:, :, 0:64],
                        pst[:].rearrange("p (two d) -> p two d", two=2),
                    )

                # ---- attention ----
                yT = apool.tile([DLOC, S], F32R, tag="yT")
                for t in range(NT):
                    psy = [ps_y.tile([65, ST], F32, tag="y", name=f"y_{b}_{t}_{h}")
                           for h in range(2)]
                    for u in range(4 * t + 4):
                        diag = u >= 4 * t
                        rel = (u - 4 * t) * 128 if diag else 0
                        na = ST - rel
                        # both heads' scores in one 2-bank psum tile; K=64
                        # matmuls at base-partition 0/64 row-tile concurrently
                        pss = ps_sc.tile([128, 2, ST], F32, tag="sc")
                        for h in range(2):
                            hs = slice(64 * h, 64 * h + 64)
                            nc.tensor.matmul(
                                pss[:, h, :na],
                                kT[hs, u * 128:(u + 1) * 128],
                                qT[hs, t * ST + rel:(t + 1) * ST],
                                start=True, stop=True,
                            )
                        at = attpool.tile([128, 2, ST], F32R, tag="at")
                        nc.scalar.activation(
                            at[:, :, :na], pss[:, :, :na],
                            mybir.ActivationFunctionType.Exp, scale=0.125,
                        )
                        if diag:
                            nc.vector.tensor_tensor(
                                at[:, :, 0:128], at[:, :, 0:128],
                                mask[:, None, :].to_broadcast((128, 2, 128)),
                                mybir.AluOpType.mult,
                            )
                        for h in range(2):
                            nc.tensor.matmul(
                                psy[h][:, rel:], v_sb[:, u, 65 * h:65 * h + 65],
                                at[:, h, :na],
                                start=(u == 0), stop=(u == 4 * t + 3),
                            )
                    for h in range(2):
                        den = spool.tile([1, ST], F32, tag="den")
                        nc.vector.tensor_copy(den[:], psy[h][64:65, :])
                        dbc = spool.tile([64, ST], F32, tag="dbc")
                        nc.gpsimd.partition_broadcast(dbc[:], den[0:1, :])
                        rcp = spool.tile([64, ST], F32, tag="rcp")
                        nc.vector.reciprocal(rcp[:], dbc[:])
                        nc.vector.tensor_tensor(
                            yT[64 * h:64 * h + 64, t * ST:(t + 1) * ST],
                            psy[h][0:64, :], rcp[:], mybir.AluOpType.mult,
                        )

                    # ---- out projection for this sq tile ----
                    for si in range(4):
                        s0 = t * ST + si * 128
                        for eo2 in range(2):
                            pso = ps_mm.tile([128, 512], F32, tag="mm")
                            nc.tensor.matmul(
                                pso[:], yT[:, s0:s0 + 128],
                                wo[:, eo2 * 512:(eo2 + 1) * 512],
                                start=True, stop=True,
                            )
                            ob = opool.tile([128, 512], F32, tag="ob")
                            eng = nc.vector if (si + eo2) % 2 == 0 else nc.scalar
                            if eng is nc.vector:
                                nc.vector.tensor_copy(ob[:], pso[:])
                            else:
                                nc.scalar.activation(
                                    ob[:], pso[:],
                                    mybir.ActivationFunctionType.Copy,
                                )
                            nc.sync.dma_start(
                                out_d[b, s0:s0 + 128, eo2 * 512:(eo2 + 1) * 512],
                                ob[:],
                            )


def _get_compiled(loop_n=1):
    if loop_n not in _compiled:
        _compiled[loop_n] = _build(loop_n)
    return _compiled[loop_n]


def _prep_inputs(x, Wq, Wk, Wv, Wo):
    xT = np.ascontiguousarray(x.transpose(0, 2, 1), dtype=np.float32)
    mask = np.triu(np.ones((128, 128), np.float32))  # mask[p, j] = 1 iff j >= p
    in_maps = []
    for c in range(NCORES):
        r = slice(128 * c, 128 * c + 128)
        in_maps.append({
            "xT": xT,
            "wqT": np.ascontiguousarray(Wq[r, :].T, dtype=np.float32),
            "wkT": np.ascontiguousarray(Wk[r, :].T, dtype=np.float32),
            "wvT": np.ascontiguousarray(Wv[r, :].T, dtype=np.float32),
            "woT": np.ascontiguousarray(Wo[:, r].T, dtype=np.float32),
            "mask": mask,
        })
    return in_maps


def kernel(x, Wq, Wk, Wv, Wo):
    from concourse.bass_utils import run_bass_kernel_spmd

    nc = _get_compiled()
    in_maps = _prep_inputs(np.asarray(x), np.asarray(Wq), np.asarray(Wk),
                           np.asarray(Wv), np.asarray(Wo))
    res = run_bass_kernel_spmd(nc, in_maps, core_ids=list(range(NCORES)))
    out = np.zeros((B, S, E), np.float32)
    for c in range(NCORES):
        out += res.results[c]["out"]
    return out


# revision 18
# speedup vs baseline: 206.9403x; 1.1063x over previous
"""Causal self-attention (B=4, S=2048, E=1024, H=16, Dh=64) on 8 TRN2 NeuronCores.

Sharding: tensor-parallel over heads — core c owns heads {2c, 2c+1}, i.e. rows
[128c, 128c+128) of the Q/K/V projection output and the matching 128 input rows
of the out-projection. Each core computes a full-shape partial output; the host
sums the 8 partials (the all-reduce after out_proj).

Layout strategy on device (all matmuls in float32r — full-rate, ~1e-4 relative
error): everything is kept "feature-major" (the contraction dim on partitions)
so no fp32 transposes of activations are ever needed:
  qT/kT [d, s] <- wT-slices @ xT ;  scoresT [sk, sq] = kT.T-chunks @ qT ;
  attnT = exp(scoresT/8) * causal ;  yT_ext [65, sq] = [v | 1].T @ attnT
(the appended ones-column yields the softmax denominator in row 64);
out[s, e] = (yT/den).T @ woT  with woT pre-transposed on the host.
"""
import sys
import os

sys.path.insert(0, "/opt/trn_rl_repo")
os.environ.setdefault("JAX_COMPILATION_CACHE_DIR", "/tmp/jax_cache")

import numpy as np

B, S, E = 4, 2048, 1024
H, DH = 16, 64
NCORES = 8
DLOC = E // NCORES          # 128 head-dims per core (2 heads)
ST = 512                    # sq tile width
SKC = 128                   # sk chunk (psum partition dim)
NT = S // ST                # 4 sq tiles
NU = S // SKC               # 16 sk chunks
EO = E // 128               # 8 contraction chunks for projections

_compiled = {}

DEFAULT_CFG = dict(xsplit=2, mm_bufs=2, sc_bufs=2, y_bufs=2, at_bufs=8,
                   op_tag="y", op_bufs=0, x_bufs=3, vt_bufs=1, sp_bufs=2)


def _build(loop_n=1, cfg=None):
    cfg = {**DEFAULT_CFG, **(cfg or {})}
    import concourse.bacc as bacc
    import concourse.mybir as mybir
    import concourse.tile as tile
    from concourse.masks import make_identity

    F32R = mybir.dt.float32r
    F32 = mybir.dt.float32

    nc = bacc.Bacc("TRN2", target_bir_lowering=False, debug=False)

    xT_d = nc.dram_tensor("xT", [B, E, S], F32R, kind="ExternalInput")
    wqT_d = nc.dram_tensor("wqT", [E, DLOC], F32R, kind="ExternalInput")
    wkT_d = nc.dram_tensor("wkT", [E, DLOC], F32R, kind="ExternalInput")
    wvT_d = nc.dram_tensor("wvT", [E, DLOC], F32R, kind="ExternalInput")
    woT_d = nc.dram_tensor("woT", [DLOC, E], F32R, kind="ExternalInput")
    mask_d = nc.dram_tensor("mask", [128, 128], F32R, kind="ExternalInput")
    out_d = nc.dram_tensor("out", [B, S, E], F32, kind="ExternalOutput")

    with tile.TileContext(nc) as tc:
        with (
            tc.tile_pool(name="const", bufs=1) as cpool,
            tc.tile_pool(name="xp", bufs=cfg["x_bufs"]) as xpool,
            tc.tile_pool(name="actp", bufs=2) as apool,
            tc.tile_pool(name="vtp", bufs=cfg["vt_bufs"]) as vtpool,
            tc.tile_pool(name="attp", bufs=cfg["at_bufs"]) as attpool,
            tc.tile_pool(name="smallp", bufs=cfg["sp_bufs"]) as spool,
            tc.tile_pool(name="outp", bufs=4) as opool,
            tc.tile_pool(name="ps_mm", bufs=cfg["mm_bufs"], space="PSUM") as ps_mm,
            tc.tile_pool(name="ps_sc", bufs=cfg["sc_bufs"], space="PSUM") as ps_sc,
            tc.tile_pool(name="ps_y", bufs=cfg["y_bufs"], space="PSUM") as ps_y,
        ):
            # ---- persistent constants ----
            wq = cpool.tile([128, EO, DLOC], F32R, tag="wq")
            wk = cpool.tile([128, EO, DLOC], F32R, tag="wk")
            wv = cpool.tile([128, EO, DLOC], F32R, tag="wv")
            wo = cpool.tile([128, E], F32R, tag="wo")
            mask = cpool.tile([128, 128], F32R, tag="mask")
            nc.sync.dma_start(wq[:], wqT_d.rearrange("(eo p) d -> p eo d", p=128))
            nc.sync.dma_start(wk[:], wkT_d.rearrange("(eo p) d -> p eo d", p=128))
            nc.sync.dma_start(wv[:], wvT_d.rearrange("(eo p) d -> p eo d", p=128))
            nc.sync.dma_start(wo[:], woT_d[:])
            nc.sync.dma_start(mask[:], mask_d[:])
            ident32 = cpool.tile([128, 128], F32, tag="id32")
            make_identity(nc, ident32[:])
            ident = cpool.tile([128, 128], F32R, tag="id")
            nc.vector.tensor_copy(ident[:], ident32[:])
            ones16 = cpool.tile([128, NU], F32, tag="ones16")
            nc.vector.memset(ones16[:], 1.0)

            import contextlib
            loop_ctx = tc.For_i(0, loop_n, 1) if loop_n > 1 else contextlib.nullcontext()
            with loop_ctx:
                _emit_body(nc, tc, locals(), cfg)

    nc.compile()
    return nc


def _emit_body(nc, tc, env, cfg):
    import concourse.mybir as mybir

    F32R = mybir.dt.float32r
    F32 = mybir.dt.float32
    (xT_d, out_d, xpool, apool, attpool, spool, opool, ps_mm, ps_sc, ps_y,
     wq, wk, wv, wo, mask, ident, ones16) = (
        env["xT_d"], env["out_d"], env["xpool"], env["apool"], env["attpool"],
        env["spool"], env["opool"], env["ps_mm"], env["ps_sc"], env["ps_y"],
        env["wq"], env["wk"], env["wv"], env["wo"], env["mask"], env["ident"],
        env["ones16"])
    vtpool = env["vtpool"]
    op_pool, op_tag = (ps_y, "y") if cfg["op_tag"] == "y" else (ps_mm, "mm")
    if True:
            for b in range(B):
                xs = cfg["xsplit"]
                xr = xT_d[b].rearrange("(eo p) s -> p eo s", p=128)

                # ---- projections (Dh-major), streaming x in ST-wide chunks ----
                qT = apool.tile([DLOC, S], F32R, tag="qT")
                kT = apool.tile([DLOC, S], F32R, tag="kT")
                vT = vtpool.tile([DLOC, S], F32R, tag="vT")
                for st in range(NT):
                    xc = xpool.tile([128, EO, ST], F32R, tag="xc")
                    for xi in range(xs):
                        lo, hi = xi * EO // xs, (xi + 1) * EO // xs
                        nc.gpsimd.dma_start(
                            xc[:, lo:hi], xr[:, lo:hi, st * ST:(st + 1) * ST])
                    for w_sb, dst in ((wq, qT), (wk, kT), (wv, vT)):
                        ps = ps_mm.tile([128, ST], F32, tag="mm")
                        for eo in range(EO):
                            nc.tensor.matmul(
                                ps[:], w_sb[:, eo], xc[:, eo],
                                start=(eo == 0), stop=(eo == EO - 1),
                            )
                        nc.vector.tensor_copy(dst[:, st * ST:(st + 1) * ST], ps[:])

                # ---- V -> S-major with ones columns ----
                # v_sb[:, u, 0:64] = head0, col 64 = 1; [65:129] = head1, col 129 = 1
                v_sb = apool.tile([128, NU, 130], F32R, tag="v")
                nc.vector.tensor_copy(v_sb[:, :, 64], ones16[:])
                nc.vector.tensor_copy(v_sb[:, :, 129], ones16[:])
                for u in range(NU):
                    pst = ps_mm.tile([128, 128], F32R, tag="mm")
                    nc.tensor.transpose(pst[:], vT[:, u * 128:(u + 1) * 128], ident[:])
                    # one strided copy: [128, 2, 64] view of both head halves
                    nc.vector.tensor_copy(
                        v_sb[:, u].rearrange("p (two d) -> p two d", two=2)[:, :, 0:64],
                        pst[:].rearrange("p (two d) -> p two d", two=2),
                    )

                # ---- attention ----
                yT = apool.tile([DLOC, S], F32R, tag="yT")
                for t in range(NT):
                    psy = [ps_y.tile([65, ST], F32, tag="y", name=f"y_{b}_{t}_{h}")
                           for h in range(2)]
                    for u in range(4 * t + 4):
                        diag = u >= 4 * t
                        rel = (u - 4 * t) * 128 if diag else 0
                        na = ST - rel
                        # both heads' scores in one 2-bank psum tile; K=64
                        # matmuls at base-partition 0/64 row-tile concurrently
                        pss = ps_sc.tile([128, 2, ST], F32, tag="sc")
                        for h in range(2):
                            hs = slice(64 * h, 64 * h + 64)
                            nc.tensor.matmul(
                                pss[:, h, :na],
                                kT[hs, u * 128:(u + 1) * 128],
                                qT[hs, t * ST + rel:(t + 1) * ST],
                                start=True, stop=True,
                            )
                        at = attpool.tile([128, 2, ST], F32R, tag="at")
                        nc.scalar.activation(
                            at[:, :, :na], pss[:, :, :na],
                            mybir.ActivationFunctionType.Exp, scale=0.125,
                        )
                        if diag:
                            nc.vector.tensor_tensor(
                                at[:, :, 0:128], at[:, :, 0:128],
                                mask[:, None, :].to_broadcast((128, 2, 128)),
                                mybir.AluOpType.mult,
                            )
                        for h in range(2):
                            nc.tensor.matmul(
                                psy[h][:, rel:], v_sb[:, u, 65 * h:65 * h + 65],
                                at[:, h, :na],
                                start=(u == 0), stop=(u == 4 * t + 3),
                            )
                    for h in range(2):
                        # free the psum slot early: one [65,512] copy, then
                        # normalize SBUF-side (2x DVE mode applies there)
                        ye = spool.tile([65, ST], F32, tag="ye")
                        nc.vector.tensor_copy(ye[:], psy[h][:])
                        dbc = spool.tile([64, ST], F32, tag="dbc")
                        nc.gpsimd.partition_broadcast(dbc[:], ye[64:65, :])
                        rcp = spool.tile([64, ST], F32, tag="rcp")
                        nc.vector.reciprocal(rcp[:], dbc[:])
                        nc.vector.tensor_tensor(
                            yT[64 * h:64 * h + 64, t * ST:(t + 1) * ST],
                            ye[0:64, :], rcp[:], mybir.AluOpType.mult,
                        )

                    # ---- out projection, deferred one sq-tile so its psum
                    # deps are long satisfied when PE reaches these matmuls
                    if t > 0:
                        _emit_outproj(nc, cfg, env, b, t - 1, yT)
                _emit_outproj(nc, cfg, env, b, NT - 1, yT)


def _emit_outproj(nc, cfg, env, b, t, yT):
    import concourse.mybir as mybir
    F32 = mybir.dt.float32
    out_d, opool, ps_mm, ps_y, wo = (
        env["out_d"], env["opool"], env["ps_mm"], env["ps_y"], env["wo"])
    op_pool, op_tag = (ps_y, "y") if cfg["op_tag"] == "y" else (ps_mm, "mm")
    for si in range(4):
        s0 = t * ST + si * 128
        for eo2 in range(2):
            pso = op_pool.tile([128, 512], F32, tag=op_tag)
            nc.tensor.matmul(
                pso[:], yT[:, s0:s0 + 128],
                wo[:, eo2 * 512:(eo2 + 1) * 512],
                start=True, stop=True,
            )
            ob = opool.tile([128, 512], F32, tag="ob")
            if (si + eo2) % 2 == 0:
                nc.vector.tensor_copy(ob[:], pso[:])
            else:
                nc.scalar.activation(
                    ob[:], pso[:], mybir.ActivationFunctionType.Copy,
                )
            nc.sync.dma_start(
                out_d[b, s0:s0 + 128, eo2 * 512:(eo2 + 1) * 512], ob[:],
            )


def _get_compiled(loop_n=1, cfg=None):
    key = (loop_n, tuple(sorted((cfg or {}).items())))
    if key not in _compiled:
        _compiled[key] = _build(loop_n, cfg)
    return _compiled[key]


def _prep_inputs(x, Wq, Wk, Wv, Wo):
    xT = np.ascontiguousarray(x.transpose(0, 2, 1), dtype=np.float32)
    mask = np.triu(np.ones((128, 128), np.float32))  # mask[p, j] = 1 iff j >= p
    in_maps = []
    for c in range(NCORES):
        r = slice(128 * c, 128 * c + 128)
        in_maps.append({
            "xT": xT,
            "wqT": np.ascontiguousarray(Wq[r, :].T, dtype=np.float32),
            "wkT": np.ascontiguousarray(Wk[r, :].T, dtype=np.float32),
            "wvT": np.ascontiguousarray(Wv[r, :].T, dtype=np.float32),
            "woT": np.ascontiguousarray(Wo[:, r].T, dtype=np.float32),
            "mask": mask,
        })
    return in_maps


def kernel(x, Wq, Wk, Wv, Wo):
    from concourse.bass_utils import run_bass_kernel_spmd

    nc = _get_compiled()
    in_maps = _prep_inputs(np.asarray(x), np.asarray(Wq), np.asarray(Wk),
                           np.asarray(Wv), np.asarray(Wo))
    res = run_bass_kernel_spmd(nc, in_maps, core_ids=list(range(NCORES)))
    out = np.zeros((B, S, E), np.float32)
    for c in range(NCORES):
        out += res.results[c]["out"]
    return out


# revision 20
# speedup vs baseline: 235.4836x; 1.1379x over previous
"""Causal self-attention (B=4, S=2048, E=1024, H=16, Dh=64) on 8 TRN2 NeuronCores.

Sharding: tensor-parallel over heads — core c owns heads {2c, 2c+1}, i.e. rows
[128c, 128c+128) of the Q/K/V projection output and the matching 128 input rows
of the out-projection. Each core computes a full-shape partial output; the host
sums the 8 partials (the all-reduce after out_proj).

Layout strategy on device (all matmuls in float32r — full-rate, ~1e-4 relative
error): everything is kept "feature-major" (the contraction dim on partitions)
so no fp32 transposes of activations are ever needed:
  qT/kT [d, s] <- wT-slices @ xT ;  scoresT [sk, sq] = kT.T-chunks @ qT ;
  attnT = exp(scoresT/8) * causal ;  yT_ext [65, sq] = [v | 1].T @ attnT
(the appended ones-column yields the softmax denominator in row 64);
out[s, e] = (yT/den).T @ woT  with woT pre-transposed on the host.
"""
import sys
import os

sys.path.insert(0, "/opt/trn_rl_repo")
os.environ.setdefault("JAX_COMPILATION_CACHE_DIR", "/tmp/jax_cache")

import numpy as np

B, S, E = 4, 2048, 1024
H, DH = 16, 64
NCORES = 8
DLOC = E // NCORES          # 128 head-dims per core (2 heads)
ST = 512                    # sq tile width
SKC = 128                   # sk chunk (psum partition dim)
NT = S // ST                # 4 sq tiles
NU = S // SKC               # 16 sk chunks
EO = E // 128               # 8 contraction chunks for projections

_compiled = {}

DEFAULT_CFG = dict(xsplit=2, mm_bufs=2, sc_bufs=2, y_bufs=2, at_bufs=8,
                   op_tag="y", op_bufs=0, x_bufs=3, vt_bufs=1, sp_bufs=2)


def _build(loop_n=1, cfg=None):
    cfg = {**DEFAULT_CFG, **(cfg or {})}
    import concourse.bacc as bacc
    import concourse.mybir as mybir
    import concourse.tile as tile
    from concourse.masks import make_identity

    F32R = mybir.dt.float32r
    F32 = mybir.dt.float32

    nc = bacc.Bacc("TRN2", target_bir_lowering=False, debug=False)

    xT_d = nc.dram_tensor("xT", [B, E, S], F32R, kind="ExternalInput")
    wqT_d = nc.dram_tensor("wqT", [E, DLOC], F32R, kind="ExternalInput")
    wkT_d = nc.dram_tensor("wkT", [E, DLOC], F32R, kind="ExternalInput")
    wvT_d = nc.dram_tensor("wvT", [E, DLOC], F32R, kind="ExternalInput")
    woT_d = nc.dram_tensor("woT", [DLOC, E], F32R, kind="ExternalInput")
    mask_d = nc.dram_tensor("mask", [128, 128], F32R, kind="ExternalInput")
    out_d = nc.dram_tensor("out", [B, S, E], F32, kind="ExternalOutput")

    with tile.TileContext(nc) as tc:
        with (
            tc.tile_pool(name="const", bufs=1) as cpool,
            tc.tile_pool(name="xp", bufs=cfg["x_bufs"]) as xpool,
            tc.tile_pool(name="actp", bufs=2) as apool,
            tc.tile_pool(name="vtp", bufs=cfg["vt_bufs"]) as vtpool,
            tc.tile_pool(name="attp", bufs=cfg["at_bufs"]) as attpool,
            tc.tile_pool(name="smallp", bufs=cfg["sp_bufs"]) as spool,
            tc.tile_pool(name="outp", bufs=4) as opool,
            tc.tile_pool(name="ps_mm", bufs=cfg["mm_bufs"], space="PSUM") as ps_mm,
            tc.tile_pool(name="ps_sc", bufs=cfg["sc_bufs"], space="PSUM") as ps_sc,
            tc.tile_pool(name="ps_y", bufs=cfg["y_bufs"], space="PSUM") as ps_y,
        ):
            # ---- persistent constants ----
            wq = cpool.tile([128, EO, DLOC], F32R, tag="wq")
            wk = cpool.tile([128, EO, DLOC], F32R, tag="wk")
            wv = cpool.tile([128, EO, DLOC], F32R, tag="wv")
            wo = cpool.tile([128, E], F32R, tag="wo")
            mask = cpool.tile([128, 128], F32R, tag="mask")
            nc.sync.dma_start(wq[:], wqT_d.rearrange("(eo p) d -> p eo d", p=128))
            nc.sync.dma_start(wk[:], wkT_d.rearrange("(eo p) d -> p eo d", p=128))
            nc.sync.dma_start(wv[:], wvT_d.rearrange("(eo p) d -> p eo d", p=128))
            nc.sync.dma_start(wo[:], woT_d[:])
            nc.sync.dma_start(mask[:], mask_d[:])
            ident32 = cpool.tile([128, 128], F32, tag="id32")
            make_identity(nc, ident32[:])
            ident = cpool.tile([128, 128], F32R, tag="id")
            nc.vector.tensor_copy(ident[:], ident32[:])
            ones16 = cpool.tile([128, NU], F32, tag="ones16")
            nc.vector.memset(ones16[:], 1.0)

            import contextlib
            loop_ctx = tc.For_i(0, loop_n, 1) if loop_n > 1 else contextlib.nullcontext()
            with loop_ctx:
                _emit_body(nc, tc, locals(), cfg)

    nc.compile()
    return nc


def _emit_body(nc, tc, env, cfg):
    import concourse.mybir as mybir

    F32R = mybir.dt.float32r
    F32 = mybir.dt.float32
    (xT_d, out_d, xpool, apool, attpool, spool, opool, ps_mm, ps_sc, ps_y,
     wq, wk, wv, wo, mask, ident, ones16) = (
        env["xT_d"], env["out_d"], env["xpool"], env["apool"], env["attpool"],
        env["spool"], env["opool"], env["ps_mm"], env["ps_sc"], env["ps_y"],
        env["wq"], env["wk"], env["wv"], env["wo"], env["mask"], env["ident"],
        env["ones16"])
    vtpool = env["vtpool"]
    op_pool, op_tag = (ps_y, "y") if cfg["op_tag"] == "y" else (ps_mm, "mm")
    if True:
            for b in range(B):
                xs = cfg["xsplit"]
                xr = xT_d[b].rearrange("(eo p) s -> p eo s", p=128)

                # ---- projections (Dh-major), streaming x in ST-wide chunks ----
                qT = apool.tile([DLOC, S], F32R, tag="qT")
                kT = apool.tile([DLOC, S], F32R, tag="kT")
                vT = vtpool.tile([DLOC, S], F32R, tag="vT")
                for st in range(NT):
                    xc = xpool.tile([128, EO, ST], F32R, tag="xc")
                    for xi in range(xs):
                        lo, hi = xi * EO // xs, (xi + 1) * EO // xs
                        nc.gpsimd.dma_start(
                            xc[:, lo:hi], xr[:, lo:hi, st * ST:(st + 1) * ST])
                    for w_sb, dst in ((wq, qT), (wk, kT), (wv, vT)):
                        ps = ps_mm.tile([128, ST], F32, tag="mm")
                        for eo in range(EO):
                            nc.tensor.matmul(
                                ps[:], w_sb[:, eo], xc[:, eo],
                                start=(eo == 0), stop=(eo == EO - 1),
                            )
                        nc.vector.tensor_copy(dst[:, st * ST:(st + 1) * ST], ps[:])

                # ---- V -> S-major with ones columns ----
                # v_sb[:, u, 0:64] = head0 v, col 64 = 1; 65:129 = head1, 129 = 1
                v_sb = apool.tile([128, NU, 130], F32R, tag="v")
                nc.vector.tensor_copy(v_sb[:, :, 64], ones16[:])
                nc.vector.tensor_copy(v_sb[:, :, 129], ones16[:])
                for u in range(NU):
                    pst = ps_mm.tile([128, 128], F32R, tag="mm")
                    nc.tensor.transpose(pst[:], vT[:, u * 128:(u + 1) * 128], ident[:])
                    # one strided copy: [128, 2, 64] view of both head halves
                    nc.vector.tensor_copy(
                        v_sb[:, u].rearrange("p (two d) -> p two d", two=2)[:, :, 0:64],
                        pst[:].rearrange("p (two d) -> p two d", two=2),
                    )

                # ---- attention ----
                yT = apool.tile([DLOC, S], F32R, tag="yT")
                for t in range(NT):
                    psy = [ps_y.tile([65, ST], F32, tag="y", name=f"y_{b}_{t}_{h}")
                           for h in range(2)]
                    for u in range(4 * t + 4):
                        diag = u >= 4 * t
                        rel = (u - 4 * t) * 128 if diag else 0
                        na = ST - rel
                        # both heads' scores in one 2-bank psum tile; K=64
                        # matmuls at base-partition 0/64 row-tile concurrently
                        pss = ps_sc.tile([128, 2, ST], F32, tag="sc")
                        for h in range(2):
                            hs = slice(64 * h, 64 * h + 64)
                            nc.tensor.matmul(
                                pss[:, h, :na],
                                kT[hs, u * 128:(u + 1) * 128],
                                qT[hs, t * ST + rel:(t + 1) * ST],
                                start=True, stop=True,
                            )
                        at = attpool.tile([128, 2, ST], F32R, tag="at")
                        nc.scalar.activation(
                            at[:, :, :na], pss[:, :, :na],
                            mybir.ActivationFunctionType.Exp, scale=0.125,
                        )
                        if diag:
                            nc.vector.tensor_tensor(
                                at[:, :, 0:128], at[:, :, 0:128],
                                mask[:, None, :].to_broadcast((128, 2, 128)),
                                mybir.AluOpType.mult,
                            )
                        for h in range(2):
                            nc.tensor.matmul(
                                psy[h][:, rel:], v_sb[:, u, 65 * h:65 * h + 65],
                                at[:, h, :na],
                                start=(u == 0), stop=(u == 4 * t + 3),
                            )
                    for h in range(2):
                        # free the psum slot early: one [65,512] copy, then
                        # normalize SBUF-side (2x DVE mode applies there)
                        ye = spool.tile([65, ST], F32, tag="ye")
                        nc.vector.tensor_copy(ye[:], psy[h][:])
                        den01 = spool.tile([1, ST], F32, tag="den01")
                        nc.vector.tensor_copy(den01[:], ye[64:65, :])
                        dbc = spool.tile([64, ST], F32, tag="dbc")
                        nc.gpsimd.partition_broadcast(dbc[:], den01[0:1, :])
                        rcp = spool.tile([64, ST], F32, tag="rcp")
                        nc.vector.reciprocal(rcp[:], dbc[:])
                        nc.vector.tensor_tensor(
                            yT[64 * h:64 * h + 64, t * ST:(t + 1) * ST],
                            ye[0:64, :], rcp[:], mybir.AluOpType.mult,
                        )

                    # ---- out projection, deferred one sq-tile so its psum
                    # deps are long satisfied when PE reaches these matmuls
                    if t > 0:
                        _emit_outproj(nc, cfg, env, b, t - 1, yT)
                _emit_outproj(nc, cfg, env, b, NT - 1, yT)


def _emit_outproj(nc, cfg, env, b, t, yT):
    import concourse.mybir as mybir
    F32 = mybir.dt.float32
    out_d, opool, ps_mm, ps_y, wo = (
        env["out_d"], env["opool"], env["ps_mm"], env["ps_y"], env["wo"])
    op_pool, op_tag = (ps_y, "y") if cfg["op_tag"] == "y" else (ps_mm, "mm")
    for si in range(4):
        s0 = t * ST + si * 128
        for eo2 in range(2):
            pso = op_pool.tile([128, 512], F32, tag=op_tag)
            nc.tensor.matmul(
                pso[:], yT[:, s0:s0 + 128],
                wo[:, eo2 * 512:(eo2 + 1) * 512],
                start=True, stop=True,
            )
            ob = opool.tile([128, 512], F32, tag="ob")
            if (si + eo2) % 2 == 0:
                nc.vector.tensor_copy(ob[:], pso[:])
            else:
                nc.scalar.activation(
                    ob[:], pso[:], mybir.ActivationFunctionType.Copy,
                )
            nc.sync.dma_start(
                out_d[b, s0:s0 + 128, eo2 * 512:(eo2 + 1) * 512], ob[:],
            )


def _get_compiled(loop_n=1, cfg=None):
    key = (loop_n, tuple(sorted((cfg or {}).items())))
    if key not in _compiled:
        _compiled[key] = _build(loop_n, cfg)
    return _compiled[key]


def _prep_inputs(x, Wq, Wk, Wv, Wo):
    xT = np.ascontiguousarray(x.transpose(0, 2, 1), dtype=np.float32)
    mask = np.triu(np.ones((128, 128), np.float32))  # mask[p, j] = 1 iff j >= p
    in_maps = []
    for c in range(NCORES):
        r = slice(128 * c, 128 * c + 128)
        in_maps.append({
            "xT": xT,
            "wqT": np.ascontiguousarray(Wq[r, :].T, dtype=np.float32),
            "wkT": np.ascontiguousarray(Wk[r, :].T, dtype=np.float32),
            "wvT": np.ascontiguousarray(Wv[r, :].T, dtype=np.float32),
            "woT": np.ascontiguousarray(Wo[:, r].T, dtype=np.float32),
            "mask": mask,
        })
    return in_maps


def kernel(x, Wq, Wk, Wv, Wo):
    from concourse.bass_utils import run_bass_kernel_spmd

    nc = _get_compiled()
    in_maps = _prep_inputs(np.asarray(x), np.asarray(Wq), np.asarray(Wk),
                           np.asarray(Wv), np.asarray(Wo))
    res = run_bass_kernel_spmd(nc, in_maps, core_ids=list(range(NCORES)))
    out = np.zeros((B, S, E), np.float32)
    for c in range(NCORES):
        out += res.results[c]["out"]
    return out
